# revision 5
# baseline (speedup 1.0000x reference)
"""Adaptive Gaussian Bilateral Filter (nn_AGBF) — Trainium2 Bass kernel, 8 NeuronCores.

Problem: a tiny two-layer attention net predicts per-patch (16x16) sigmas
(sx, sy, sr); a k x k bilateral filter (k data-dependent, k=11 for the given
inputs) is applied to a [2,1,512,512] image with reflect padding.

Sharding: 8 cores x 128 image rows (4 cores per batch element). Each core:
  - runs the sigma net for its batch redundantly (attention over all 1024
    patches; queries restricted to its own 256 patches via host-side column
    permutation of the patch matrix, so the SPMD program is core-independent),
  - computes the bilateral filter for its 256 blocks in a blocks-on-partitions
    layout [128 blocks, 256 pixels] (2 chunks): sigma is constant per block,
    so the spatial term folds into the per-partition bias of the ACT exp and
    the range scale folds into a precomputed U = x/(sqrt(2)*sr) image.
  - per tap: DVE d=(Uc-Un), sq=d*d (bf16, taps batched per dy/parity);
    ACT w=exp(-sq-cxy) per tap; DVE m=w*Un; PE identity-matmuls accumulate
    S0=sum w, S1=sum m into PSUM.  out = (S1*sqrt(2)*sr)/(S0+1e-8).
"""

import math
import os
import sys
from functools import lru_cache

import numpy as np

sys.path.insert(0, "/opt/trn_rl_repo")

PS = 16
HD = 8
B, C, H, W = 2, 1, 512, 512
NCORES = 8
RPC = (B * H) // NCORES          # 128 rows per core
BRC = RPC // PS                  # 8 block-rows per core
WB = W // PS                     # 32 block-cols
NBLK = BRC * WB                  # 256 blocks per core
PIX = PS * PS                    # 256 pixels per block
NP_ = (H // PS) * (W // PS)      # 1024 patches per batch
DIN = C * PS * PS                # 256


# ----------------------------------------------------------------------------
# host-side sigma-net mirror (numpy) — used only to pick the data-dependent k
# ----------------------------------------------------------------------------

def _np_softmax(s):
    s = s - s.max(-1, keepdims=True)
    e = np.exp(s)
    return e / e.sum(-1, keepdims=True)


def _np_attn(q, k, v):
    s = np.einsum('bnd,bmd->bnm', q, k) * (HD ** -0.5)
    return np.einsum('bnm,bmd->bnd', _np_softmax(s), v)


def _np_patch_sigmas(x, Wq, bq, Wk, bk, Wv, bv, Wsq, bsq, Wsk, bsk, Wsv, bsv,
                     ln_g, ln_b, Wp, bp):
    b, c, h, w = x.shape
    hp, wp = h // PS, w // PS
    p = x.reshape(b, c, hp, PS, wp, PS).transpose(0, 2, 4, 1, 3, 5).reshape(b, hp * wp, c * PS * PS)
    feats = _np_attn(p @ Wq + bq, p @ Wk + bk, p @ Wv + bv)
    sp = _np_attn(feats @ Wsq + bsq, feats @ Wsk + bsk, feats @ Wsv + bsv)
    mu = sp.mean(-1, keepdims=True)
    var = ((sp - mu) ** 2).mean(-1, keepdims=True)
    sp = (sp - mu) / np.sqrt(var + 1e-5) * ln_g + ln_b
    sp = sp @ Wp + bp
    sp = np.minimum(np.log1p(np.exp(sp)), 6.0) + 1e-6
    return sp  # [B, NP, 3]


def _pick_k(inputs):
    sp = _np_patch_sigmas(**{k: np.asarray(v) for k, v in inputs.items()})
    m = float(max(sp[..., 0].max(), sp[..., 1].max()))
    k = int(2 * math.ceil(m) + 1)
    if k % 2 == 0:
        k += 1
    return k


# ----------------------------------------------------------------------------
# device kernel builder
# ----------------------------------------------------------------------------

def _split_multi_waits(nc, mybir):
    """This container's walrus accepts only ONE sync wait per instruction;
    hoist extra waits onto inserted wait-only NoOps on the same engine."""
    for f in nc.m.functions:
        for blk in f.blocks:
            new_insts = []
            for inst in blk.instructions:
                si = inst.sync_info
                if si is not None and si.on_wait and len(si.on_wait) > 1:
                    extra, keep = si.on_wait[:-1], si.on_wait[-1:]
                    for j, wt in enumerate(extra):
                        nop = mybir.InstNoOp(
                            name=f"{inst.name}-ws{j}", ins=[], outs=[],
                            sync_info=mybir.SyncInfo(on_wait=[wt], on_update=[]))
                        nop.engine = inst.engine
                        new_insts.append(nop)
                    si.on_wait[:] = keep
                new_insts.append(inst)
            blk.instructions[:] = new_insts


@lru_cache(maxsize=4)
def _build(k):
    import concourse.bass as bass
    import concourse.tile as tile
    from concourse import mybir
    from concourse.mybir import AluOpType as Alu

    F32 = mybir.dt.float32
    BF16 = mybir.dt.bfloat16
    Act = mybir.ActivationFunctionType

    p = k // 2
    WIN = PS + 2 * p            # window side (even)
    WELEM = WIN * WIN
    CTR = p * WIN + p           # center offset in window

    nc = bass.Bass()

    def par(name, shape, out=False):
        return nc.declare_dram_parameter(name, list(shape), F32, isOutput=out)

    xh_in = par('xh', [NBLK, WELEM])
    pt_in = par('pt', [DIN, NP_])
    wq_in = par('Wq', [DIN, HD]); bq_in = par('bq', [HD, 1])
    wk_in = par('Wk', [DIN, HD]); bk_in = par('bk', [HD, 1])
    wv_in = par('Wv', [DIN, HD]); bv_in = par('bv', [1, HD])
    wsq_in = par('Wsq', [HD, HD]); bsq_in = par('bsq', [HD, 1])
    wsk_in = par('Wsk', [HD, HD]); bsk_in = par('bsk', [HD, 1])
    wsv_in = par('Wsv', [HD, HD]); bsv_in = par('bsv', [1, HD])
    lng_in = par('ln_g', [1, HD]); lnb_in = par('ln_b', [1, HD])
    wp_in = par('Wp', [HD, 3]); bp_in = par('bp', [3, 1])
    dsq_in = par('dsq', [1, k])
    id_in = par('ident', [128, 128])
    out_ext = par('out', [NBLK, PIX], out=True)

    def bcast(param, n):
        ap = param[:]
        return bass.AP(tensor=ap.tensor, offset=0, ap=[[0, 128], [1, n]])

    def view(t, extra_off, dims):
        return bass.AP(tensor=t.tensor, offset=t.offset + extra_off,
                       ap=[list(t.ap[0])] + [list(d) for d in dims])

    NB2 = NP_ // 512
    HD1 = HD + 1

    with tile.TileContext(nc) as tc:
        with tc.tile_pool(name='persist', bufs=1) as pp, \
             tc.tile_pool(name='work', bufs=2) as wkp, \
             tc.tile_pool(name='et', bufs=2) as etp, \
             tc.tile_pool(name='flt', bufs=3) as fp, \
             tc.tile_pool(name='sqp', bufs=18) as sqp, \
             tc.tile_pool(name='psF', bufs=1, space='PSUM') as psF:

            # ---- image halo first: the filter d/sq sweeps depend only on it
            xh_sb = [pp.tile([128, WELEM], F32, tag=f'xh{i}', name=f'xh{i}') for i in range(2)]
            xb = [pp.tile([128, WELEM], BF16, tag=f'xb{i}', name=f'xb{i}') for i in range(2)]
            xbo = [pp.tile([128, WELEM], BF16, tag=f'xbo{i}', name=f'xbo{i}') for i in range(2)]
            for i in range(2):
                nc.sync.dma_start(out=xh_sb[i][:], in_=xh_in[128 * i:128 * i + 128, :])
                nc.vector.tensor_copy(out=xb[i][:], in_=xh_sb[i][:])
                nc.vector.tensor_copy(out=xbo[i][:, 0:WELEM - 1], in_=xb[i][:, 1:WELEM])

            s01_ps = [psF.tile([128, 2 * PIX], F32, tag=f's01_{qc}', name=f's01_{qc}')
                      for qc in range(2)]

            # sigma-independent d/sq sweeps, pumped into the attention phase
            groups = [(qc, dx) for qc in range(2) for dx in range(k)]
            sq_tiles = {}
            emitted = [0]

            def emit_dsq():
                gi = emitted[0]
                if gi >= len(groups):
                    return
                emitted[0] += 1
                qc, dx = groups[gi]
                if CTR % 2 == 0:
                    uc_t, uc_off = xb[qc], CTR
                else:
                    uc_t, uc_off = xbo[qc], CTR - 1
                ucv = view(uc_t, uc_off, [[0, k], [WIN, PS], [1, PS]])
                src, base = (xb[qc], dx) if dx % 2 == 0 else (xbo[qc], dx - 1)
                unv = view(src, base, [[WIN, k], [WIN, PS], [1, PS]])
                sq = sqp.tile([128, k * PIX], BF16, tag='sq', name=f'sq{gi}')
                nc.vector.tensor_tensor(out=sq[:], in0=ucv, in1=unv, op=Alu.subtract)
                nc.vector.tensor_tensor(out=sq[:], in0=sq[:], in1=sq[:], op=Alu.mult)
                sq_tiles[gi] = (sq, unv)

            def pump(n):
                for _ in range(n):
                    emit_dsq()

            pump(2)

            # ---- constants / weights -----------------------------------
            idf = pp.tile([128, 128], F32, tag='idf')
            nc.sync.dma_start(out=idf[:], in_=id_in[:])
            idb = pp.tile([128, 128], BF16, tag='idb')
            nc.vector.tensor_copy(out=idb[:], in_=idf[:])
            eps1 = pp.tile([128, 1], F32, tag='eps1')
            nc.vector.memset(eps1[:], 1e-5)
            one3 = pp.tile([3, 1], F32, tag='one3')
            nc.vector.memset(one3[:], 1.0)

            wq_sb = [pp.tile([128, HD], BF16, tag=f'wq{i}', name=f'wq{i}') for i in range(2)]
            wk_sb = [pp.tile([128, HD], BF16, tag=f'wk{i}', name=f'wk{i}') for i in range(2)]
            wv_sb = [pp.tile([128, HD], BF16, tag=f'wv{i}', name=f'wv{i}') for i in range(2)]
            for i in range(2):
                for (dst, src) in ((wq_sb[i], wq_in), (wk_sb[i], wk_in), (wv_sb[i], wv_in)):
                    f32tmp = wkp.tile([128, HD], F32, tag='wtmp')
                    nc.sync.dma_start(out=f32tmp[:], in_=src[128 * i:128 * i + 128, :])
                    nc.gpsimd.tensor_copy(out=dst[:], in_=f32tmp[:])
            w2_sb = {}
            for nm, src in (('wsq', wsq_in), ('wsk', wsk_in), ('wsv', wsv_in)):
                f32tmp = wkp.tile([HD, HD], F32, tag='w2tmp')
                nc.sync.dma_start(out=f32tmp[:], in_=src[:])
                t = pp.tile([HD, HD], BF16, tag=nm, name=nm)
                nc.gpsimd.tensor_copy(out=t[:], in_=f32tmp[:])
                w2_sb[nm] = t
            wsq_sb, wsk_sb, wsv_sb = w2_sb['wsq'], w2_sb['wsk'], w2_sb['wsv']
            f32tmp = wkp.tile([HD, 3], F32, tag='wptmp')
            nc.sync.dma_start(out=f32tmp[:], in_=wp_in[:])
            wp_sb = pp.tile([HD, 3], BF16, tag='wp')
            nc.gpsimd.tensor_copy(out=wp_sb[:], in_=f32tmp[:])

            bq_sb = pp.tile([HD, 1], F32, tag='bq')
            nc.sync.dma_start(out=bq_sb[:], in_=bq_in[:])
            bk_sb = pp.tile([HD, 1], F32, tag='bk')
            nc.sync.dma_start(out=bk_sb[:], in_=bk_in[:])
            bsq_sb = pp.tile([HD, 1], F32, tag='bsq')
            nc.sync.dma_start(out=bsq_sb[:], in_=bsq_in[:])
            bsk_sb = pp.tile([HD, 1], F32, tag='bsk')
            nc.sync.dma_start(out=bsk_sb[:], in_=bsk_in[:])
            bp_sb = pp.tile([3, 1], F32, tag='bp')
            nc.sync.dma_start(out=bp_sb[:], in_=bp_in[:])
            bv_b = pp.tile([128, HD], F32, tag='bv')
            nc.gpsimd.dma_start(out=bv_b[:], in_=bcast(bv_in, HD))
            bsv_b = pp.tile([128, HD], F32, tag='bsv')
            nc.gpsimd.dma_start(out=bsv_b[:], in_=bcast(bsv_in, HD))
            lng_b = pp.tile([128, HD], F32, tag='lng')
            nc.gpsimd.dma_start(out=lng_b[:], in_=bcast(lng_in, HD))
            lnb_b = pp.tile([128, HD], F32, tag='lnb')
            nc.gpsimd.dma_start(out=lnb_b[:], in_=bcast(lnb_in, HD))
            dsq_b = pp.tile([128, k], F32, tag='dsq')
            nc.gpsimd.dma_start(out=dsq_b[:], in_=bcast(dsq_in, k))

            pt_sb = [pp.tile([128, NP_], BF16, tag=f'pt{i}', name=f'pt{i}') for i in range(2)]
            for i in range(2):
                ptf = wkp.tile([128, NP_], F32, tag='ptf')
                nc.sync.dma_start(out=ptf[:], in_=pt_in[128 * i:128 * i + 128, :])
                nc.gpsimd.tensor_copy(out=pt_sb[i][:], in_=ptf[:])

            pump(2)

            # ---- attention 1 ------------------------------------------
            qT_sb = pp.tile([HD, NP_], BF16, tag='qT')
            kT_sb = pp.tile([HD, NP_], BF16, tag='kT')
            v_sb = pp.tile([128, HD1 * 8], BF16, tag='v')
            with tc.tile_pool(name='psA', bufs=2, space='PSUM') as psA:
                for (w_sb, b_sb, dst) in ((wq_sb, bq_sb, qT_sb), (wk_sb, bk_sb, kT_sb)):
                    qk_ps = psA.tile([HD, NP_], F32, tag='big')
                    for bank in range(NB2):
                        for fh in range(2):
                            nc.tensor.matmul(qk_ps[:, 512 * bank:512 * bank + 512],
                                             w_sb[fh][:], pt_sb[fh][:, 512 * bank:512 * bank + 512],
                                             start=(fh == 0), stop=(fh == 1))
                    nc.scalar.add(out=dst[:], in_=qk_ps[:], add=b_sb[:, 0:1])
                for chn in range(8):
                    v_ps = psA.tile([128, HD], F32, tag='big', name=f'v_ps{chn}')
                    for fh in range(2):
                        nc.tensor.matmul(v_ps[:], pt_sb[fh][:, 128 * chn:128 * chn + 128],
                                         wv_sb[fh][:], start=(fh == 0), stop=(fh == 1))
                    nc.vector.tensor_add(out=v_sb[:, HD1 * chn:HD1 * chn + HD], in0=v_ps[:], in1=bv_b[:, 0:HD])
                    nc.vector.memset(v_sb[:, HD1 * chn + HD:HD1 * chn + HD1], 1.0)
                pump(2)

                fT_ps = psA.tile([HD1, NP_], F32, tag='fT', bufs=1)
                for kc in range(8):
                    sT_ps = psA.tile([128, NP_], F32, tag='big', name=f'sT_ps{kc}')
                    for bank in range(NB2):
                        nc.tensor.matmul(sT_ps[:, 512 * bank:512 * bank + 512],
                                         kT_sb[:, 128 * kc:128 * kc + 128],
                                         qT_sb[:, 512 * bank:512 * bank + 512],
                                         start=True, stop=True)
                    eT = etp.tile([128, NP_], BF16, tag='eT')
                    nc.scalar.activation(out=eT[:], in_=sT_ps[:], func=Act.Exp, scale=HD ** -0.5)
                    for bank in range(NB2):
                        nc.tensor.matmul(fT_ps[:, 512 * bank:512 * bank + 512],
                                         v_sb[:, HD1 * kc:HD1 * kc + HD1],
                                         eT[:, 512 * bank:512 * bank + 512],
                                         start=(kc == 0), stop=(kc == 7), skip_group_check=True)
                    pump(1)
                fT_sb = pp.tile([HD1, NP_], F32, tag='fTs')
                nc.scalar.copy(out=fT_sb[:], in_=fT_ps[:])

            # normalize feats; den comes back via the appended ones row
            fnT_sb = pp.tile([HD, NP_], BF16, tag='fnT')
            with tc.tile_pool(name='psB', bufs=2, space='PSUM') as psB:
                for qc in range(8):
                    f_ps = psB.tile([128, HD1], F32, tag='tp', name=f'f_ps{qc}')
                    nc.tensor.transpose(f_ps[:], fT_sb[:, 128 * qc:128 * qc + 128], idf[0:HD1, 0:HD1])
                    dn_r = wkp.tile([128, 1], F32, tag='dnr')
                    nc.vector.reciprocal(out=dn_r[:], in_=f_ps[:, HD:HD1])
                    fn = wkp.tile([128, HD], F32, tag='fn')
                    nc.vector.tensor_scalar_mul(out=fn[:], in0=f_ps[:, 0:HD], scalar1=dn_r[:, 0:1])
                    fnT_ps = psB.tile([HD, 128], F32, tag='tp', name=f'fnT_ps{qc}')
                    nc.tensor.transpose(fnT_ps[:], fn[:], idf[:])
                    nc.scalar.copy(out=fnT_sb[:, 128 * qc:128 * qc + 128], in_=fnT_ps[:])
                    if qc % 3 == 2:
                        pump(1)

                # ---- attention 2 ----------------------------------------
                q2T_sb = pp.tile([HD, NBLK], BF16, tag='q2T')
                k2T_sb = pp.tile([HD, NP_], BF16, tag='k2T')
                v2_sb = pp.tile([128, HD1 * 8], BF16, tag='v2')
                q2_ps = psB.tile([HD, NBLK], F32, tag='tp')
                nc.tensor.matmul(q2_ps[:], wsq_sb[:], fnT_sb[:, 0:NBLK], start=True, stop=True)
                nc.scalar.add(out=q2T_sb[:], in_=q2_ps[:], add=bsq_sb[:, 0:1])
                for bank in range(NB2):
                    k2_ps = psB.tile([HD, 512], F32, tag='tp', name=f'k2_ps{bank}')
                    nc.tensor.matmul(k2_ps[:], wsk_sb[:], fnT_sb[:, 512 * bank:512 * bank + 512],
                                     start=True, stop=True)
                    nc.scalar.add(out=k2T_sb[:, 512 * bank:512 * bank + 512], in_=k2_ps[:], add=bsk_sb[:, 0:1])
                for chn in range(8):
                    v2_ps = psB.tile([128, HD], F32, tag='tp', name=f'v2_ps{chn}')
                    nc.tensor.matmul(v2_ps[:], fnT_sb[:, 128 * chn:128 * chn + 128], wsv_sb[:],
                                     start=True, stop=True)
                    nc.vector.tensor_add(out=v2_sb[:, HD1 * chn:HD1 * chn + HD], in0=v2_ps[:], in1=bsv_b[:, 0:HD])
                    nc.vector.memset(v2_sb[:, HD1 * chn + HD:HD1 * chn + HD1], 1.0)
                pump(1)

                spT_ps = psB.tile([HD1, NBLK], F32, tag='spT', bufs=1)
                for kc in range(8):
                    s2_ps = psB.tile([128, NBLK], F32, tag='s2', name=f's2_ps{kc}')
                    nc.tensor.matmul(s2_ps[:], k2T_sb[:, 128 * kc:128 * kc + 128], q2T_sb[:],
                                     start=True, stop=True)
                    e2 = etp.tile([128, NBLK], BF16, tag='e2')
                    nc.scalar.activation(out=e2[:], in_=s2_ps[:], func=Act.Exp, scale=HD ** -0.5)
                    nc.tensor.matmul(spT_ps[:], v2_sb[:, HD1 * kc:HD1 * kc + HD1], e2[:],
                                     start=(kc == 0), stop=(kc == 7), skip_group_check=True)
                    if kc % 3 == 2:
                        pump(1)
                spT_sb = pp.tile([HD1, NBLK], F32, tag='spTs')
                nc.scalar.copy(out=spT_sb[:], in_=spT_ps[:])

                # ---- normalize, LN, project, softplus -------------------
                sig_sb = pp.tile([3, NBLK], F32, tag='sig')
                xnT_sb = pp.tile([HD, NBLK], BF16, tag='xnT')
                for qc in range(2):
                    sl = slice(128 * qc, 128 * qc + 128)
                    sp_ps = psB.tile([128, HD1], F32, tag='tp', name=f'sp_ps{qc}')
                    nc.tensor.transpose(sp_ps[:], spT_sb[:, sl], idf[0:HD1, 0:HD1])
                    d2_r = wkp.tile([128, 1], F32, tag='dnr')
                    nc.vector.reciprocal(out=d2_r[:], in_=sp_ps[:, HD:HD1])
                    spq = wkp.tile([128, HD], F32, tag='spq')
                    nc.vector.tensor_scalar_mul(out=spq[:], in0=sp_ps[:, 0:HD], scalar1=d2_r[:, 0:1])
                    st = wkp.tile([128, nc.vector.BN_STATS_DIM], F32, tag='st')
                    nc.vector.bn_stats(out=st[:], in_=spq[:])
                    mv = wkp.tile([128, nc.vector.BN_AGGR_DIM], F32, tag='mv')
                    nc.vector.bn_aggr(out=mv[:], in_=st[:])
                    lnv = wkp.tile([128, 1], F32, tag='lnv')
                    nc.scalar.activation(out=lnv[:], in_=mv[:, 1:2], func=Act.Ln, bias=eps1[:, 0:1], scale=1.0)
                    rstd = wkp.tile([128, 1], F32, tag='rstd')
                    nc.scalar.activation(out=rstd[:], in_=lnv[:], func=Act.Exp, scale=-0.5)
                    xn = wkp.tile([128, HD], F32, tag='xn')
                    nc.vector.tensor_scalar(out=xn[:], in0=spq[:], scalar1=mv[:, 0:1], scalar2=rstd[:, 0:1],
                                            op0=Alu.subtract, op1=Alu.mult)
                    nc.vector.tensor_tensor(out=xn[:], in0=xn[:], in1=lng_b[:, 0:HD], op=Alu.mult)
                    nc.vector.tensor_tensor(out=xn[:], in0=xn[:], in1=lnb_b[:, 0:HD], op=Alu.add)
                    xnT_ps = psB.tile([HD, 128], F32, tag='tp', name=f'xnT_ps{qc}')
                    nc.tensor.transpose(xnT_ps[:], xn[:], idf[:])
                    nc.scalar.copy(out=xnT_sb[:, sl], in_=xnT_ps[:])
                lg_ps = psB.tile([3, NBLK], F32, tag='tp')
                nc.tensor.matmul(lg_ps[:], wp_sb[:], xnT_sb[:], start=True, stop=True)
                lg_sb = pp.tile([3, NBLK], F32, tag='lg')
                nc.scalar.add(out=lg_sb[:], in_=lg_ps[:], add=bp_sb[:, 0:1])
                nc.scalar.activation(out=lg_sb[:], in_=lg_sb[:], func=Act.Exp, scale=1.0)
                nc.scalar.activation(out=lg_sb[:], in_=lg_sb[:], func=Act.Ln, bias=one3[:, 0:1], scale=1.0)
                nc.vector.tensor_scalar(out=sig_sb[:], in0=lg_sb[:], scalar1=6.0, scalar2=1e-6,
                                        op0=Alu.min, op1=Alu.add)

                # ---- per-chunk filter params ----------------------------
                negal, negcx, dgy = [], [], []
                for qc in range(2):
                    sl = slice(128 * qc, 128 * qc + 128)
                    sg_ps = psB.tile([128, 3], F32, tag='tp', name=f'sg_ps{qc}')
                    nc.tensor.transpose(sg_ps[:], sig_sb[:, sl], idf[0:3, 0:3])
                    sg = pp.tile([128, 3], F32, tag=f'sg{qc}', name=f'sg{qc}')
                    nc.vector.tensor_copy(out=sg[:], in_=sg_ps[:])
                    nal = pp.tile([128, 1], F32, tag=f'nal{qc}', name=f'nal{qc}')
                    nc.vector.reciprocal(out=nal[:], in_=sg[:, 2:3])
                    nc.vector.tensor_tensor(out=nal[:], in0=nal[:], in1=nal[:], op=Alu.mult)
                    nc.vector.tensor_scalar_mul(out=nal[:], in0=nal[:], scalar1=-0.5)
                    negal.append(nal)
                    ncx = pp.tile([128, k], F32, tag=f'ncx{qc}', name=f'ncx{qc}')
                    ncy = wkp.tile([128, k], F32, tag='ncy')
                    for (ax, dst) in ((0, ncx), (1, ncy)):
                        isv = wkp.tile([128, 1], F32, tag='isv')
                        nc.vector.reciprocal(out=isv[:], in_=sg[:, ax:ax + 1])
                        nc.vector.tensor_tensor(out=isv[:], in0=isv[:], in1=isv[:], op=Alu.mult)
                        nc.vector.tensor_scalar_mul(out=isv[:], in0=isv[:], scalar1=-0.5)
                        nc.vector.tensor_scalar_mul(out=dst[:], in0=dsq_b[:, 0:k], scalar1=isv[:, 0:1])
                    negcx.append(ncx)
                    gyv = wkp.tile([128, k], F32, tag='gyv')
                    nc.scalar.activation(out=gyv[:], in_=ncy[:], func=Act.Exp, scale=1.0)
                    dg_list = []
                    for dy in range(k):
                        dg = pp.tile([128, 128], BF16, tag=f'dgy{qc}_{dy}', name=f'dgy{qc}_{dy}')
                        nc.vector.tensor_scalar_mul(out=dg[:], in0=idb[:], scalar1=gyv[:, dy:dy + 1])
                        dg_list.append(dg)
                    dgy.append(dg_list)

            # ---- bilateral filter main loop -----------------------------
            for gi, (qc, dx) in enumerate(groups):
                emit_dsq()
                sq, unv = sq_tiles.pop(gi)
                wm = fp.tile([128, k * 2 * PIX], BF16, tag='wm')
                wv_ = view(wm, 0, [[2 * PIX, k], [1, PIX]])
                nc.scalar.activation(out=wv_, in_=view(sq, 0, [[PIX, k], [1, PIX]]),
                                     func=Act.Exp, bias=negcx[qc][:, dx:dx + 1],
                                     scale=negal[qc][:, 0:1])
                nc.vector.tensor_tensor(out=view(wm, PIX, [[2 * PIX, k], [1, PIX]]),
                                        in0=wv_, in1=unv, op=Alu.mult)
                for dy in range(k):
                    nc.tensor.matmul(s01_ps[qc][:], dgy[qc][dy],
                                     wm[:, 2 * PIX * dy:2 * PIX * dy + 2 * PIX],
                                     start=(gi % k == 0 and dy == 0), stop=(gi % k == k - 1 and dy == k - 1),
                                     skip_group_check=True)

            # ---- finalize: out = S1 / (S0 + 1e-8) -----------------------
            for qc in range(2):
                den = fp.tile([128, PIX], F32, tag='fden')
                nc.vector.tensor_scalar_add(out=den[:], in0=s01_ps[qc][:, 0:PIX], scalar1=1e-8)
                nc.vector.reciprocal(out=den[:], in_=den[:])
                res = fp.tile([128, PIX], F32, tag='fres')
                nc.vector.tensor_tensor(out=res[:], in0=s01_ps[qc][:, PIX:2 * PIX], in1=den[:], op=Alu.mult)
                nc.sync.dma_start(out=out_ext[128 * qc:128 * qc + 128, :], in_=res[:])

    _split_multi_waits(nc, mybir)
    return nc


# ----------------------------------------------------------------------------
# host glue
# ----------------------------------------------------------------------------

def _prep_inputs(inputs, k):
    p = k // 2
    WIN = PS + 2 * p
    x = np.ascontiguousarray(np.asarray(inputs['x'], dtype=np.float32))
    xpad = np.pad(x[:, 0], ((0, 0), (p, p), (p, p)), mode='reflect')

    shared = {}
    for nm in ('Wq', 'Wk', 'Wv', 'Wsq', 'Wsk', 'Wsv', 'Wp'):
        shared[nm] = np.ascontiguousarray(np.asarray(inputs[nm], dtype=np.float32))
    for nm in ('bq', 'bk', 'bsq', 'bsk', 'bp'):
        shared[nm] = np.asarray(inputs[nm], dtype=np.float32).reshape(-1, 1)
    for nm in ('bv', 'bsv', 'ln_g', 'ln_b'):
        shared[nm] = np.asarray(inputs[nm], dtype=np.float32).reshape(1, -1)
    offs = np.arange(-p, p + 1, dtype=np.float32)
    shared['dsq'] = (offs ** 2).reshape(1, -1)
    shared['ident'] = np.eye(128, dtype=np.float32)

    in_maps = []
    for c in range(NCORES):
        b, s = divmod(c, NCORES // B)
        slab = xpad[b, RPC * s: RPC * s + RPC + 2 * p, :]
        winv = np.lib.stride_tricks.sliding_window_view(slab, (WIN, WIN))[::PS, ::PS]
        xh = np.ascontiguousarray(winv.reshape(NBLK, WIN * WIN), dtype=np.float32)
        xb = x[b, 0]
        pt = xb.reshape(H // PS, PS, W // PS, PS).transpose(1, 3, 0, 2).reshape(DIN, NP_)
        lo = NBLK * s
        perm = np.r_[lo:lo + NBLK, 0:lo, lo + NBLK:NP_]
        pt = np.ascontiguousarray(pt[:, perm], dtype=np.float32)
        m = dict(shared)
        m['xh'] = xh
        m['pt'] = pt
        in_maps.append(m)
    return in_maps


def _assemble(results):
    out = np.empty((B, C, H, W), dtype=np.float32)
    for c in range(NCORES):
        b, s = divmod(c, NCORES // B)
        r = results[c]['out']  # [NBLK, PIX]
        blk = r.reshape(BRC, WB, PS, PS).transpose(0, 2, 1, 3).reshape(RPC, W)
        out[b, 0, RPC * s: RPC * s + RPC, :] = blk
    return out


def kernel(**inputs):
    from concourse.bass_utils import run_bass_kernel_spmd
    k = _pick_k(inputs)
    nc = _build(k)
    in_maps = _prep_inputs(inputs, k)
    res = run_bass_kernel_spmd(nc, in_maps, core_ids=list(range(NCORES)),
                               trace=bool(int(os.environ.get('AGBF_TRACE', '0'))))
    out = _assemble(res.results)
    if os.environ.get('AGBF_TRACE', '0') != '0':
        kernel.last_exec_time_ns = res.exec_time_ns
    return out
